# revision 26
# baseline (speedup 1.0000x reference)
"""Multi-head attention (B=2, S=2048, D=1024, 16 heads x 64) on 8 TRN2 cores.

Sharding: 2 batches x 4 head-groups. Core c owns batch c//4 and heads
{4*(c%4) .. 4*(c%4)+3} = rows [256*(c%4), 256*(c%4)+256) of Wq/Wk/Wv;
it computes its (S, 256) slice of that batch's context. No collectives;
host assembles the (B, S, D) output.

Per-core pipeline (matmul operands bf16, f32 PSUM accumulation):
  All transposes run on the XBAR DMA engine (dma_start_transpose,
  out[j,c,i] = in[i, 128c+j]) in BATCHED instructions (13 total), issued
  from the SP/sync queue so the ACT sequencer never blocks; the PE only
  does matmuls. x, W: f32 load -> DVE cast bf16 -> XBAR -> xT/wT.
  The exp of the full score matrix on ACT (~1.1ns/elem -> ~143us) is the
  hard floor; the schedule keeps ACT saturated from ~25us: prelude
  projects kT0, vT0 (+em fold: v2 carries em[t] col so PV emits the
  softmax denominator Z), qT0 seg0, then attention streams QK->exp->PV
  per (head, 512-block, t-chunk-pair), PV lagging one pair, while
  remaining projection matmuls pump into the PE's ACT-paced slack.
  Epilogue per head: DVE copy h[80,512] bf16, one XBAR -> [s,16,80],
  1/Z scale, SWDGE store. PSUM: proj 2x[128,512] + scores 2x[128,2,512]
  + ph 2x[80,512] = 16KB.
"""

import sys

if "/opt/trn_rl_repo" not in sys.path:
    sys.path.insert(0, "/opt/trn_rl_repo")

import numpy as np

B = 2
S = 2048
D = 1024
NCORES = 8
WC = 256          # per-core projection width (4 heads x 64)
HEADS = 4         # heads per core
W = 64            # head dim
PW = 80           # padded per-head width in v2/ph (W + Z col + zeros)
NWC = WC // 128   # w-chunks (2)
KC = D // 128     # contraction chunks (8)
SC = S // 128     # 128-row chunks of S (16)
SEG = 512         # matmul moving-dim segment
NSEG = S // SEG   # 4
SBLK = 512        # attention s-block
NBLK = S // SBLK  # 4
TG = 2            # t-chunks per exp group (PSUM tile = 2 banks)
PUMP = 2          # proj matmuls interleaved per attention group


def _build():
    from collections import deque

    import concourse.bass as bass
    import concourse.tile as tile
    from concourse import bacc, mybir
    from concourse.masks import make_identity

    f32 = mybir.dt.float32
    bf16 = mybir.dt.bfloat16
    EXP = mybir.ActivationFunctionType.Exp

    nc = bacc.Bacc("TRN2", target_bir_lowering=False, debug=False)

    x_d = nc.dram_tensor("hidden_states", [S, D], f32, kind="ExternalInput")
    m_d = nc.dram_tensor("attn_mask", [S], f32, kind="ExternalInput")
    wq_d = nc.dram_tensor("wq", [WC, D], f32, kind="ExternalInput")
    wk_d = nc.dram_tensor("wk", [WC, D], f32, kind="ExternalInput")
    wv_d = nc.dram_tensor("wv", [WC, D], f32, kind="ExternalInput")
    bq_d = nc.dram_tensor("bq", [WC], f32, kind="ExternalInput")
    bk_d = nc.dram_tensor("bk", [WC], f32, kind="ExternalInput")
    bv_d = nc.dram_tensor("bv", [WC], f32, kind="ExternalInput")
    o_d = nc.dram_tensor("out", [S, WC], f32, kind="ExternalOutput")

    with tile.TileContext(nc) as tc:
        consts = tc.alloc_tile_pool(name="consts", bufs=1)
        xfp = tc.alloc_tile_pool(name="xfp", bufs=4)
        xbp = tc.alloc_tile_pool(name="xbp", bufs=4)
        wfp = tc.alloc_tile_pool(name="wfp", bufs=1)
        wsp = tc.alloc_tile_pool(name="wsp", bufs=1)
        xtp = tc.alloc_tile_pool(name="xtp", bufs=1)
        qkp = tc.alloc_tile_pool(name="qkp", bufs=1)
        vp = tc.alloc_tile_pool(name="vp", bufs=1)
        etp = tc.alloc_tile_pool(name="etp", bufs=6)
        hp = tc.alloc_tile_pool(name="hp", bufs=2)
        ptp = tc.alloc_tile_pool(name="ptp", bufs=2)
        op = tc.alloc_tile_pool(name="op", bufs=2)
        ps_pr = tc.alloc_tile_pool(name="ps_pr", bufs=2, space="PSUM")
        ps_sc = tc.alloc_tile_pool(name="ps_sc", bufs=2, space="PSUM")
        ps_h = tc.alloc_tile_pool(name="ps_h", bufs=2, space="PSUM")

        # --- mask -> em[t] = exp(1e4*m - 1e4), laid out [t_local, t_chunk] ---
        mb = consts.tile([128, 1], f32, tag="mbias")
        nc.vector.memset(mb[:, :], -10000.0)
        msk = consts.tile([128, SC], f32, tag="mask")
        nc.gpsimd.dma_start(out=msk[:, :], in_=m_d.ap().rearrange("(c p) -> p c", p=128))
        em = consts.tile([128, SC], f32, tag="em")
        nc.scalar.activation(em[:, :], msk[:, :], EXP, scale=10000.0, bias=mb[:, :])

        identb = consts.tile([128, 128], bf16, tag="identb")
        make_identity(nc, identb[:, :])

        def transpose8(dst, src, chunks=KC, copy_eng=None):
            """PE-transpose `chunks` 128x128 bf16 blocks of `src` packed into
            one PSUM tile (bf16 view), then one copy into dst (DVE, or ACT
            during the prelude where the Scalar engine is otherwise idle)."""
            ptf = ps_pr.tile([128, SEG], f32, tag="pr", name="tr")
            pt = ptf[:, :].bitcast(bf16).rearrange("p (a b) -> p a b", b=128)
            for j in range(chunks):
                nc.tensor.transpose(
                    pt[:, j, :], src[:, j * 128:(j + 1) * 128], identb[:, :]
                )
            if copy_eng is nc.scalar:
                nc.scalar.copy(dst, pt[:, 0:chunks, :])
            else:
                nc.vector.tensor_copy(dst, pt[:, 0:chunks, :])

        # --- weights: f32 load -> DVE cast bf16 -> PE transpose ---
        wts = {}
        for name, wd in (("q", wq_d), ("k", wk_d), ("v", wv_d)):
            wt = consts.tile([128, NWC, KC, 128], bf16, tag=f"wt_{name}")
            for wc in range(NWC):
                wf = wfp.tile([128, D], f32, tag=f"wf_{name}{wc}")
                nc.scalar.dma_start(out=wf[:, :], in_=wd[wc * 128:(wc + 1) * 128, :])
                wb = wsp.tile([128, D], bf16, tag=f"w_{name}{wc}")
                nc.vector.tensor_copy(wb[:, :], wf[:, :])
                transpose8(wt[:, wc, :, :], wb)
            wts[name] = wt

        bias = {}
        for name, bd in (("q", bq_d), ("k", bk_d), ("v", bv_d)):
            bc = consts.tile([128, NWC], f32, tag=f"b_{name}")
            nc.gpsimd.dma_start(
                out=bc[:, :], in_=bd.ap().rearrange("(wc p) -> p wc", p=128)
            )
            bias[name] = bc

        # --- xT [d, s] bf16: loads on 3 queues -> DVE cast -> PE transpose ---
        xt = xtp.tile([128, KC, S], bf16, tag="xt")
        xload = [nc.sync, nc.gpsimd, nc.scalar]

        def x_chunk(sc):
            xf = xfp.tile([128, D], f32, tag="xf")
            xload[sc % 3].dma_start(out=xf[:, :], in_=x_d[sc * 128:(sc + 1) * 128, :])
            xb = xbp.tile([128, D], bf16, tag="x")
            nc.vector.tensor_copy(xb[:, :], xf[:, :])
            transpose8(xt[:, :, sc * 128:(sc + 1) * 128], xb,
                       copy_eng=nc.scalar if sc % 2 == 0 else nc.vector)

        qt = qkp.tile([128, NWC, S], bf16, tag="qt")
        kt = qkp.tile([128, NWC, S], bf16, tag="kt")
        vt = qkp.tile([128, NWC, S], bf16, tag="vt")
        v2 = vp.tile([128, SC, HEADS, PW], bf16, tag="v2")
        nc.vector.memset(v2[:, :, :, W:W + 1], 1.0)
        nc.vector.memset(v2[:, :, :, W + 1:PW], 0.0)

        v2ready = {0: False, 1: False}

        def v_prep(wc):
            """XBAR vT straight into v2[:, :, h, 0:W] (one instr per head),
            then fold the mask in-place: v2[:, sc, h, :] *= em[sc] (the Z
            column starts at 1.0 so it becomes em, keeping Z exact)."""
            for h in (2 * wc, 2 * wc + 1):
                nc.sync.dma_start_transpose(
                    v2[:, :, h, 0:W], vt[(h % 2) * W:(h % 2 + 1) * W, wc, :]
                )
            for sc in range(SC):
                nc.vector.tensor_scalar(
                    out=v2[:, sc, 2 * wc:2 * wc + 2, :],
                    in0=v2[:, sc, 2 * wc:2 * wc + 2, :],
                    scalar1=em[:, sc:sc + 1],
                    scalar2=None,
                    op0=mybir.AluOpType.mult,
                )
            v2ready[wc] = True

        class ProjSeg:
            """One projection segment: 8 kc-matmuls into PSUM + bias-add,
            emittable one matmul at a time so it can fill PE slack."""

            def __init__(self, dst, wname, wc, sg):
                self.dst, self.wname, self.wc, self.sg = dst, wname, wc, sg
                self.kc = 0
                self.pp = None

            def step(self):
                if self.pp is None:
                    self.pp = ps_pr.tile([128, SEG], f32, tag="pr",
                                         name=f"pp_{self.wname}")
                nc.tensor.matmul(
                    self.pp[:, :],
                    lhsT=wts[self.wname][:, self.wc, self.kc, :],
                    rhs=xt[:, self.kc, self.sg * SEG:(self.sg + 1) * SEG],
                    start=(self.kc == 0),
                    stop=(self.kc == KC - 1),
                )
                self.kc += 1
                if self.kc == KC:
                    nc.vector.tensor_scalar_add(
                        self.dst[:, self.wc, self.sg * SEG:(self.sg + 1) * SEG],
                        self.pp[:, :],
                        bias[self.wname][:, self.wc:self.wc + 1],
                    )
                    return True
                return False

        pending = deque()
        vseg_left = {0: NSEG, 1: NSEG}

        def on_seg_done(seg):
            if seg.wname == "v":
                vseg_left[seg.wc] -= 1
                if vseg_left[seg.wc] == 0:
                    v_prep(seg.wc)

        def pump(n):
            while n > 0 and pending:
                seg = pending[0]
                if seg.step():
                    pending.popleft()
                    on_seg_done(seg)
                n -= 1

        def project_now(dst, wname, wc, sgs=None):
            for sg in (range(NSEG) if sgs is None else sgs):
                seg = ProjSeg(dst, wname, wc, sg)
                while not seg.step():
                    pass
                on_seg_done(seg)

        def attention(h):
            """One head: QK -> exp stream; PV pairs queue up and flush (2 per
            slot) once v2 for this head is ready, so head 0 can start before
            the V projection lands; proj matmuls pump into leftover slack.
            Epilogue once per head."""
            wc, hr = h // 2, (h % 2) * W
            pvq = deque()

            def make_pv(ph, t0, et):
                def emit():
                    for j in range(TG):
                        nc.tensor.matmul(
                            ph[:, :],
                            lhsT=v2[:, t0 + j, h, :],
                            rhs=et[:, j, :],
                            start=(t0 == 0 and j == 0),
                            stop=(t0 == SC - TG and j == TG - 1),
                        )
                return ("pv", emit)

            def make_copy(ph, hsb, blk):
                def emit():
                    nc.vector.tensor_copy(
                        hsb[:, blk * SBLK:(blk + 1) * SBLK], ph[:, :]
                    )
                return ("copy", emit)

            def flush(budget):
                while pvq:
                    kind, emit = pvq[0]
                    if kind == "pv":
                        if budget <= 0 or not v2ready[wc]:
                            return
                        budget -= 1
                    pvq.popleft()
                    emit()

            hsb = hp.tile([PW, S], bf16, tag="hsb")
            slot = 0
            for blk in range(NBLK):
                ph = ps_h.tile([PW, SBLK], f32, tag="ph")
                for t0 in range(0, SC, TG):
                    slot += 1
                    psc = ps_sc.tile([128, TG, SEG], f32, tag="sc")
                    for j in range(TG):
                        nc.tensor.matmul(
                            psc[:, j, :],
                            lhsT=kt[hr:hr + W, wc, (t0 + j) * 128:(t0 + j + 1) * 128],
                            rhs=qt[hr:hr + W, wc, blk * SBLK:(blk + 1) * SBLK],
                            start=True,
                            stop=True,
                        )
                    et = etp.tile([128, TG, SEG], bf16, tag="et")
                    nc.scalar.activation(et[:, :, :], psc[:, :, :], EXP, scale=0.125)
                    pvq.append(make_pv(ph, t0, et))
                    if h > 0 or slot > 4:
                        flush(1 if len(pvq) <= 2 else 3)
                    pump(PUMP)
                pvq.append(make_copy(ph, hsb, blk))
            while pvq:
                flush(100)
            epilogue(h, hsb)

        def epilogue(h, hsb):
            """hsb [80, S] bf16 -> one XBAR -> [s, sc, 80] -> 1/Z -> store."""
            pt = ptp.tile([128, SC, PW], bf16, tag="pt")
            nc.sync.dma_start_transpose(pt[:, :, :], hsb[:, :])
            rec = op.tile([128, SC, 1], f32, tag="rec")
            nc.vector.reciprocal(rec[:, :, :], pt[:, :, W:W + 1])
            ot = op.tile([128, SC, W], f32, tag="ot")
            for c in range(SC):
                nc.vector.tensor_scalar_mul(ot[:, c, :], pt[:, c, 0:W], rec[:, c, :])
            nc.gpsimd.dma_start(
                out=o_d[:, h * W:(h + 1) * W].rearrange("(c p) w -> p c w", p=128),
                in_=ot[:, :, :],
            )

        # prelude: x chunks with kT0 segs interleaved, then vT0, qT0 seg0 —
        # everything else pumps into the ACT-paced attention slack.
        for sg in range(NSEG):
            for j in range(4):
                x_chunk(sg * 4 + j)
            project_now(kt, "k", 0, sgs=[sg])
        project_now(qt, "q", 0, sgs=[0])
        project_now(vt, "v", 0)
        pending.extend(ProjSeg(qt, "q", 0, sg) for sg in (1, 2, 3))
        pending.extend(ProjSeg(kt, "k", 1, sg) for sg in range(NSEG))
        pending.extend(ProjSeg(vt, "v", 1, sg) for sg in range(NSEG))
        pending.extend(ProjSeg(qt, "q", 1, sg) for sg in range(NSEG))

        for h in range(HEADS):
            attention(h)
            if h == HEADS - 1:
                while pending:
                    pump(1000)

        for p in (ps_h, ps_sc, ps_pr, op, ptp, hp, etp, vp, qkp, xtp, wsp,
                  wfp, xbp, xfp, consts):
            p.release()

    nc.finalize()
    return nc


_NC = None


def _get_nc():
    global _NC
    if _NC is None:
        _NC = _build()
    return _NC


def _in_maps(inputs):
    x = np.ascontiguousarray(np.asarray(inputs["hidden_states"], dtype=np.float32))
    m = np.ascontiguousarray(np.asarray(inputs["attn_mask"], dtype=np.float32))
    maps = []
    for c in range(NCORES):
        b, hg = c // 4, c % 4
        sl = slice(hg * WC, (hg + 1) * WC)
        maps.append({
            "hidden_states": np.ascontiguousarray(x[b]),
            "attn_mask": np.ascontiguousarray(m[b]),
            "wq": np.ascontiguousarray(np.asarray(inputs["Wq"], dtype=np.float32)[sl]),
            "wk": np.ascontiguousarray(np.asarray(inputs["Wk"], dtype=np.float32)[sl]),
            "wv": np.ascontiguousarray(np.asarray(inputs["Wv"], dtype=np.float32)[sl]),
            "bq": np.ascontiguousarray(np.asarray(inputs["bq"], dtype=np.float32)[sl]),
            "bk": np.ascontiguousarray(np.asarray(inputs["bk"], dtype=np.float32)[sl]),
            "bv": np.ascontiguousarray(np.asarray(inputs["bv"], dtype=np.float32)[sl]),
        })
    return maps


def _run(inputs, trace=False):
    from concourse.bass_utils import run_bass_kernel_spmd

    nc = _get_nc()
    res = run_bass_kernel_spmd(
        nc, _in_maps(inputs), core_ids=list(range(NCORES)), trace=trace
    )
    out = np.empty((B, S, D), dtype=np.float32)
    for c in range(NCORES):
        b, hg = c // 4, c % 4
        out[b, :, hg * WC:(hg + 1) * WC] = res.results[c]["out"]
    return out, res


def kernel(**inputs):
    out, _ = _run(inputs, trace=False)
    return out
